# revision 67
# baseline (speedup 1.0000x reference)
"""2-layer GCN (DBPnet GCN head) on 8 Trainium2 NeuronCores.

Algorithm (matches the jax reference):
    x0 = relu(x)
    x1 = relu(gcn_conv(x0, W1, b1))
    x2 = gcn_conv(x1, W2, b2)
    y  = softmax(x2, axis=-1)
with gcn_conv(x) = D^-1/2 (A + I) D^-1/2 (x @ W) + b  (in-degree over dst + 1).

Sharding: nodes row-partitioned over 8 cores (6250 each); edges partitioned
by destination core so the segment-sum is core-local.  Per layer each core
computes hs = dinv * (x_shard @ W), all-gathers hs into a full bf16 table,
gathers hs[src] rows for its (dst-sorted) edges with batched indirect DMAs,
and segment-sums each 128-destination window on the tensor engine using a
one-hot selection matrix S (S[e, j] = dst_slot[e] == j) accumulated in PSUM:
    psum_w = sum_tiles S_tile^T @ gathered_tile
    out_w  = act( dinv_w * (psum_w + hs_w) + b )
The one-hot decomposition dinv[src]*dinv[dst] = (dinv applied pre-allgather)
* (dinv applied post-aggregation) makes the per-edge norm free.
"""

import sys

import numpy as np

sys.path.insert(0, "/opt/trn_rl_repo")

import ml_dtypes  # noqa: E402
from concourse import bass, mybir  # noqa: E402
import concourse.bacc as bacc  # noqa: E402
import concourse.tile as tile  # noqa: E402
from concourse.bass_utils import run_bass_kernel_spmd  # noqa: E402

F32 = mybir.dt.float32
BF16 = mybir.dt.bfloat16
I32 = mybir.dt.int32

C = 8            # cores
P = 128          # partitions / edge-tile size / window size
TG = 8           # edge tiles per gather DMA (dma_gather num_idxs = TG*128;
                 # >1024 idxs per call crashes the exec unit (1536 verified
                 # to fail with an INTERNAL runtime error) - keep <=1024
TB = 8           # edge tiles per S-build op
NQ = 4           # SWDGE queues to spread gathers over (ucode max 4)
PAD_SLOT = 200.0  # dst_slot value for padding edges (no iota match)
AGC = 4           # all-gather chunks for the layer-2 table
def _nch(N):
    """number of 128-node chunks in the (padded) full node range."""
    return (N + P - 1) // P


def _ka(N):
    """chunks in the first (pass-0) half of the layer-1 table."""
    return (_nch(N) + 1) // 2


def _agc_bounds(N):
    """window-range row bounds for the chunked layer-2 all-gather.
    Front-loaded: early chunks are big (they overlap the layer-1 tail),
    the last is tiny so little all-gather work remains after layer-1."""
    NS = N // C
    W = (NS + P - 1) // P
    fr = [0.0, 0.42, 0.75, 0.94, 1.0][:AGC + 1]
    fr[-1] = 1.0
    ws = [round(W * f) for f in fr]
    return [min(w * P, NS) for w in ws]


# ---------------------------------------------------------------- host prep

def _schedule(src, dst, N):
    """Static tile schedule, shared by all cores.

    The per-layer tile sequence is [pass 0: windows 0..W-1][pass 1: ...],
    pass h covering edges with src in half h (int16-safe local indices).
    Returns (off2 [2, W+1] global tile offsets, T_total, per_core) where
    per_core[c] = (idx_wrapped [P, T_total*8] int16, dst_slot [P, T] bf16).
    """
    NS = N // C
    W = (NS + P - 1) // P
    NCH = _nch(N)
    KA = _ka(N)
    KB = NCH - KA

    order = np.argsort(dst, kind="stable")  # sorts by core then window
    s_dst = dst[order]
    s_src = src[order]

    core_bounds = np.searchsorted(s_dst, np.arange(C + 1) * NS)
    counts = np.zeros((2, C, W), dtype=np.int64)
    cores_edges = []
    for c in range(C):
        lo, hi = core_bounds[c], core_bounds[c + 1]
        d_loc = (s_dst[lo:hi] - c * NS).astype(np.int64)
        # rotated per-core table space: own shard first (matches the
        # locally-computed hs1 table layout).  The table is split into
        # two per-pass DRAM tiles (chunks [0,KA) / [KA,NCH)), each laid
        # out partition-major: node at rotated pos r=(k*128+p) lives at
        # row p*KA + k of its half (k counted within the half).
        r = (s_src[lo:hi].astype(np.int64) - c * NS) % N
        k_e = r // P
        p_e = r % P
        h_e = (k_e >= KA).astype(np.int64)
        sc = np.where(h_e == 0, p_e * KA + k_e, p_e * KB + (k_e - KA))
        w_e = d_loc >> 7
        for h in range(2):
            m = h_e == h
            counts[h, c] = np.bincount(w_e[m], minlength=W)
        cores_edges.append((sc, d_loc, w_e, h_e))

    # tiles per (pass, window), shared across cores; >=1 so psum is zeroed
    T_hw = np.maximum(1, (counts.max(axis=1) + P - 1) // P)  # [2, W]
    off2 = np.zeros((2, W + 1), dtype=np.int64)
    off2[0, 1:] = np.cumsum(T_hw[0])
    off2[1, 0] = off2[0, -1]
    off2[1, 1:] = off2[0, -1] + np.cumsum(T_hw[1])
    T_total = int(off2[1, -1])

    per_core = []
    for c in range(C):
        sc, d_loc, w_e, h_e = cores_edges[c]
        si = np.zeros(T_total * P, np.int16)
        sl = np.full(T_total * P, PAD_SLOT, np.float32)
        for h in range(2):
            m = h_e == h
            d_h, w_h, s_h = d_loc[m], w_e[m], sc[m]
            w_start = np.concatenate([[0], np.cumsum(counts[h, c])])
            rank = np.arange(len(d_h)) - w_start[w_h]
            pos = off2[h, w_h] * P + rank
            si[pos] = s_h.astype(np.int16)
            sl[pos] = (d_h & 127).astype(np.float32)
        # dma_gather index wrap: idx i -> partition i%16, col i//16,
        # replicated across the 8 groups of 16 partitions
        siw = np.ascontiguousarray(
            np.tile(si.reshape(T_total * 8, 16).T, (8, 1)))
        sl = np.ascontiguousarray(
            sl.reshape(T_total, P).T.astype(ml_dtypes.bfloat16))
        per_core.append((siw, sl))
    return off2, T_total, per_core


def _schedule_l2(src, dst, N):
    """Layer-2 tile schedule: single pass, tiles parity-pure in src.

    The layer-2 table packs node pairs into one 256B bf16 row, so the
    gather index is flatrow>>1 (int16-safe in one pass) and flatrow&1
    picks the 64-wide half of the gathered row at matmul time, where
    flatrow is the node's row in the chunk-major local table assembled
    from the AGC chunked all-gathers.  Per window the tile sequence is
    [even tiles][odd tiles]; PAR[t] gives each tile's parity.
    Returns (offw [W+1], PAR [T2], T2, per_core [(siw, sl)]).
    """
    NS = N // C
    W = (NS + P - 1) // P
    ab = _agc_bounds(N)

    # node g = c*NS + j, with j in chunk k's range [ab[k], ab[k+1]) ->
    # flatrow = C*ab[k] + c*(ab[k+1]-ab[k]) + (j - ab[k])
    def flatrow(g):
        c_n = g // NS
        j = g % NS
        k = np.searchsorted(np.asarray(ab), j, side="right") - 1
        k = np.clip(k, 0, AGC - 1)
        a = np.asarray(ab)[k]
        b = np.asarray(ab)[k + 1]
        return C * a + c_n * (b - a) + (j - a)

    order = np.argsort(dst, kind="stable")
    s_dst = dst[order]
    s_src = src[order]

    core_bounds = np.searchsorted(s_dst, np.arange(C + 1) * NS)
    counts = np.zeros((2, C, W), dtype=np.int64)
    cores_edges = []
    for c in range(C):
        lo, hi = core_bounds[c], core_bounds[c + 1]
        d_loc = (s_dst[lo:hi] - c * NS).astype(np.int64)
        sc = flatrow(s_src[lo:hi].astype(np.int64))
        p_e = sc & 1
        w_e = d_loc >> 7
        for h in range(2):
            counts[h, c] = np.bincount(w_e[p_e == h], minlength=W)
        cores_edges.append((sc, d_loc, w_e, p_e))

    T_pw = (counts.max(axis=1) + P - 1) // P          # [2, W], may be 0
    zero = T_pw.sum(axis=0) == 0
    T_pw[0, zero] = 1                                  # keep psum chain alive
    offw = np.zeros(W + 1, dtype=np.int64)
    offw[1:] = np.cumsum(T_pw.sum(axis=0))
    T2 = int(offw[-1])
    PAR = np.zeros(T2, dtype=np.int64)
    for w in range(W):
        PAR[offw[w] + T_pw[0, w]:offw[w + 1]] = 1

    per_core = []
    for c in range(C):
        sc, d_loc, w_e, p_e = cores_edges[c]
        si = np.zeros(T2 * P, np.int16)
        sl = np.full(T2 * P, PAD_SLOT, np.float32)
        for h in range(2):
            m = p_e == h
            d_h, w_h, s_h = d_loc[m], w_e[m], sc[m]
            w_start = np.concatenate([[0], np.cumsum(counts[h, c])])
            rank = np.arange(len(d_h)) - w_start[w_h]
            base = offw[w_h] + h * T_pw[0, w_h]
            pos = base * P + rank
            si[pos] = (s_h >> 1).astype(np.int16)
            sl[pos] = (d_h & 127).astype(np.float32)
        siw = np.ascontiguousarray(np.tile(si.reshape(T2 * 8, 16).T, (8, 1)))
        sl = np.ascontiguousarray(
            sl.reshape(T2, P).T.astype(ml_dtypes.bfloat16))
        per_core.append((siw, sl))
    return offw, PAR, T2, per_core



# ------------------------------------------------------------- device build

I16 = mybir.dt.int16


def build_program(nc, N, H, F1, F2, off2, offw2, PAR2, hasb1, hasb2,
                  dbg=False, cc=True):
    """Emit the SPMD program. All cores run identical code; per-core data
    comes in through the input tensors."""
    NS = N // C
    W = (NS + P - 1) // P
    NSP = W * P
    T_total = int(off2[1, -1])
    T2 = int(offw2[-1])
    KA = _ka(N)
    KB = _nch(N) - KA
    AB = _agc_bounds(N)

    NCH = _nch(N)
    NPAD = NCH * P

    # ---- I/O -------------------------------------------------------------
    # xT holds the FULL node range, rotated so this core's shard comes
    # first; every core redundantly computes the whole layer-1 table
    # locally, which removes the layer-1 all-gather entirely.
    d_xT = nc.dram_tensor("xT", [H, NPAD], BF16, kind="ExternalInput")
    d_W1 = nc.dram_tensor("W1", [H, F1], BF16, kind="ExternalInput")
    d_dinvf = nc.dram_tensor("dinvf", [P, NCH], F32, kind="ExternalInput")
    d_W2 = nc.dram_tensor("W2", [F1, F2], BF16, kind="ExternalInput")
    d_b1 = nc.dram_tensor("b1r", [P, F1], F32, kind="ExternalInput")
    d_b2 = nc.dram_tensor("b2r", [P, F2], F32, kind="ExternalInput")
    d_dinv = nc.dram_tensor("dinv", [P, W], F32, kind="ExternalInput")
    d_dinvr = nc.dram_tensor("dinvr", [1, NSP], F32, kind="ExternalInput")
    d_iota = nc.dram_tensor("iota", [P, P], BF16, kind="ExternalInput")
    d_ident = nc.dram_tensor("ident", [P, P], BF16, kind="ExternalInput")
    d_si = nc.dram_tensor("srcidx", [P, T_total * 8], I16,
                          kind="ExternalInput")
    d_sl = nc.dram_tensor("dstslot", [P, T_total], BF16, kind="ExternalInput")
    d_si2 = nc.dram_tensor("srcidx2", [P, T2 * 8], I16, kind="ExternalInput")
    d_sl2 = nc.dram_tensor("dstslot2", [P, T2], BF16, kind="ExternalInput")
    d_y = nc.dram_tensor("y", [NS, F2], F32, kind="ExternalOutput")
    if dbg:
        d_hs1f = nc.dram_tensor("dbg_hs1full", [N, F1], BF16,
                                kind="ExternalOutput")
        d_gath = nc.dram_tensor("dbg_gath", [P, TG * F1], BF16,
                                kind="ExternalOutput")
        d_sdbg = nc.dram_tensor("dbg_s", [P, TB * P], BF16,
                                kind="ExternalOutput")
        d_hs2d = nc.dram_tensor("dbg_hs2", [P, W * F2], F32,
                                kind="ExternalOutput")

    with tile.TileContext(nc) as tc:
        with (
            tc.tile_pool(name="const", bufs=1) as const_pool,
            tc.tile_pool(name="persist", bufs=1) as persist,
            tc.tile_pool(name="gath", bufs=8) as gath_pool,
            tc.tile_pool(name="sbuild", bufs=3) as s_pool,
            tc.tile_pool(name="winbuf", bufs=3) as win_pool,
            tc.tile_pool(name="xslab", bufs=4) as x_pool,
            tc.tile_pool(name="small", bufs=6) as small_pool,
            tc.tile_pool(name="agg", bufs=2, space="PSUM") as psum_agg,
            tc.tile_pool(name="dense", bufs=4, space="PSUM") as psum_dense,
            tc.tile_pool(name="tpose", bufs=2, space="PSUM") as psum_t,
            tc.tile_pool(name="dram", bufs=1, space="DRAM") as dram,
        ):
            # ---- constants / persistent state -----------------------------
            sb_W1 = const_pool.tile([H, F1], BF16, tag="w1")
            nc.sync.dma_start(out=sb_W1[:], in_=d_W1[:])
            sb_dinvf = const_pool.tile([P, NCH], F32, tag="dinvf")
            nc.sync.dma_start(out=sb_dinvf[:], in_=d_dinvf[:])
            sb_W2 = const_pool.tile([F1, F2], BF16, tag="w2")
            nc.sync.dma_start(out=sb_W2[:], in_=d_W2[:])
            sb_b1 = const_pool.tile([P, F1], F32, tag="b1")
            nc.sync.dma_start(out=sb_b1[:], in_=d_b1[:])
            sb_b2 = const_pool.tile([P, F2], F32, tag="b2")
            nc.sync.dma_start(out=sb_b2[:], in_=d_b2[:])
            sb_dinv = const_pool.tile([P, W], F32, tag="dinv")
            nc.sync.dma_start(out=sb_dinv[:], in_=d_dinv[:])
            sb_dinvr = const_pool.tile([1, NSP], F32, tag="dinvr")
            nc.sync.dma_start(out=sb_dinvr[:], in_=d_dinvr[:])
            sb_iota = const_pool.tile([P, P], BF16, tag="iota")
            nc.sync.dma_start(out=sb_iota[:], in_=d_iota[:])
            sb_ident = const_pool.tile([P, P], BF16, tag="ident")
            nc.sync.dma_start(out=sb_ident[:], in_=d_ident[:])
            # index tables go through other engines' DGE queues so they
            # don't serialize behind the xT slab loads on sync
            sb_si = const_pool.tile([P, T_total * 8], I16, tag="srcidx")
            nc.scalar.dma_start(out=sb_si[:], in_=d_si[:])
            sb_sl = const_pool.tile([P, T_total], BF16, tag="dstslot")
            nc.scalar.dma_start(out=sb_sl[:], in_=d_sl[:])
            sb_si2 = const_pool.tile([P, T2 * 8], I16, tag="srcidx2")
            nc.scalar.dma_start(out=sb_si2[:], in_=d_si2[:])
            sb_sl2 = const_pool.tile([P, T2], BF16, tag="dstslot2")
            nc.scalar.dma_start(out=sb_sl2[:], in_=d_sl2[:])

            sb_hs1 = persist.tile([P, W, F1], BF16, tag="hs1")
            sb_hs2 = persist.tile([P, W, F2], BF16, tag="hs2")
            # pass-0 partial window aggregates, parked in bf16
            sb_acc1 = persist.tile([P, W, F1], BF16, tag="acc1")

            # gather tables: hs1 is computed locally in full (rotated node
            # order, own shard first; partition-major rows so slab writes
            # are one DMA each).  The table is split into per-pass half
            # tiles so pass-0 gathers can start once half A is written.
            # The layer-2 table is assembled from AGC chunked all-gathers
            # and bounced into one Local tile (gathers drain faster from
            # Local than from Shared); its rows pack 2 nodes (256B).
            hs1_fA = dram.tile([P, KA, F1], BF16, tag="hs1_fA")
            hs1_fB = dram.tile([P, KB, F1], BF16, tag="hs1_fB")
            hs2_locs = [
                dram.tile([AB[k + 1] - AB[k], F2], BF16,
                          tag=f"hs2_loc{k}", name=f"hs2_loc{k}")
                for k in range(AGC)]
            hs2_fss = [
                dram.tile([C * (AB[k + 1] - AB[k]), F2], BF16,
                          tag=f"hs2_fs{k}", name=f"hs2_fs{k}",
                          addr_space="Shared")
                for k in range(AGC)]
            hs2_fl = dram.tile([N, F2], BF16, tag="hs2_fl")

            # ---- phase 1: hs1 = dinv * (relu(x) @ W1) for ALL nodes -------
            XB = 8  # node chunks per streamed xT slab

            def table_write(k0, nk, stage):
                """slab [k0, k0+nk) -> half tile(s), one DMA per span."""
                spans = []
                if k0 < KA:
                    spans.append((hs1_fA, k0, 0, min(nk, KA - k0)))
                if k0 + nk > KA:
                    j0 = max(0, KA - k0)
                    spans.append((hs1_fB, k0 + j0 - KA, j0, nk - j0))
                for tile_, col, j0, n in spans:
                    nc.sync.dma_start(out=tile_[:, col:col + n, :],
                                      in_=stage[:, j0:j0 + n, :])

            for k0 in range(0, NCH, XB):
                nk = min(XB, NCH - k0)
                xb = x_pool.tile([H, XB * P], BF16, tag="xslab")
                nc.sync.dma_start(out=xb[:, :nk * P],
                                  in_=d_xT[:, k0 * P:(k0 + nk) * P])
                nc.vector.tensor_scalar_max(xb[:, :nk * P], xb[:, :nk * P],
                                            0.0)
                stage = x_pool.tile([P, XB, F1], BF16, tag="hstage")
                for j in range(nk):
                    k = k0 + j
                    ph = psum_dense.tile([P, F1], F32, tag="dense")
                    nc.tensor.matmul(ph[:], lhsT=xb[:, j * P:(j + 1) * P],
                                     rhs=sb_W1[:], start=True, stop=True)
                    # alternate the scale+cast between DVE and ACT
                    if k % 2:
                        nc.scalar.activation(
                            stage[:, j, :], ph[:],
                            mybir.ActivationFunctionType.Copy,
                            scale=sb_dinvf[:, k:k + 1])
                    else:
                        nc.vector.tensor_scalar_mul(
                            stage[:, j, :], ph[:], sb_dinvf[:, k:k + 1])
                    if k < W:  # own shard: also keep for self-loop term
                        nc.scalar.copy(sb_hs1[:, k, :], stage[:, j, :])
                table_write(k0, nk, stage)

            # ---- edge aggregation (both layers) ---------------------------
            def edge_layer(tables, F, dt, acc_sb, out_cb):
                """Two passes (one per src half); pass 0 parks the partial
                window sums in acc_sb, pass 1 finishes and calls out_cb."""
                gts = {}
                sts = {}
                for t0 in range(0, T_total, TB):  # S batches, layer dtype dt
                    n = min(TB, T_total - t0)
                    s = s_pool.tile([P, TB, P], dt, tag="sbuild")
                    nc.vector.tensor_tensor(
                        out=s[:, :n, :],
                        in0=sb_sl[:, t0:t0 + n].to_broadcast([P, n, P]),
                        in1=sb_iota[:].rearrange(
                            "p (o n) -> p o n", o=1).to_broadcast([P, n, P]),
                        op=mybir.AluOpType.is_equal)
                    if dbg and t0 == 0 and F == F1:
                        nc.sync.dma_start(out=d_sdbg[:, :n * P],
                                          in_=s[:, :n, :])
                    sts[t0 // TB] = s
                for h in range(2):
                    p_lo, p_hi = int(off2[h, 0]), int(off2[h, -1])
                    tab = tables[h]
                    for w in range(W):
                        t0w, t1w = int(off2[h, w]), int(off2[h, w + 1])
                        pa = psum_agg.tile([P, F1], F32, tag="agg")
                        for t in range(t0w, t1w):
                            if (t - p_lo) % TG == 0:
                                g = gath_pool.tile([P, TG, F], dt, tag="gath")
                                n = min(TG, p_hi - t)
                                gi = (t - p_lo) // TG
                                nc.gpsimd.dma_gather(
                                    g[:, :n, :], tab,
                                    sb_si[:, t * 8:(t + n) * 8],
                                    n * P, n * P, F,
                                    queue_num=gi % NQ)
                                if dbg and t == 0 and F == F1:
                                    nc.sync.dma_start(
                                        out=d_gath[:, :n * F],
                                        in_=g[:, :n, :])
                                gts[(t - p_lo) // TG + 1000 * h] = g
                            nc.tensor.matmul(
                                pa[:, :F],
                                lhsT=sts[t // TB][:, t % TB, :],
                                rhs=gts[(t - p_lo) // TG + 1000 * h][
                                    :, (t - p_lo) % TG, :],
                                start=(t == t0w),
                                stop=(h == 0 and t == t1w - 1))
                        rows = min(P, NS - w * P)
                        if h == 0:
                            nc.scalar.copy(acc_sb[:, w, :], pa[:, :F])
                        else:
                            # out_cb continues the psum chain (ident adds)
                            out_cb(w, rows, pa)

            # ---- layer-1 epilogue: relu, transpose, dense L2 --------------
            # acc1/self-loop/bias terms join the psum chain on the tensor
            # engine (ident / rank-1 matmuls); scaling+relu happen on ACT
            # straight out of PSUM, leaving the vector engine to S-builds.
            def l1_out(w, rows, pa):
                nc.tensor.matmul(pa[:, :F1], lhsT=sb_ident[:],
                                 rhs=sb_acc1[:, w, :],
                                 start=False, stop=False)
                nc.tensor.matmul(pa[:, :F1], lhsT=sb_ident[:],
                                 rhs=sb_hs1[:, w, :],
                                 start=False, stop=not hasb1)
                if hasb1:  # pa += sqrt(deg) x b1 so relu(dinv*pa) is exact
                    nc.tensor.matmul(
                        pa[:, :F1],
                        lhsT=sb_dinvr[:, w * P:(w + 1) * P],
                        rhs=sb_b1[0:1, :], start=False, stop=True)
                x1 = win_pool.tile([P, F1], BF16, tag="x1")
                nc.scalar.activation(x1[:], pa[:, :F1],
                                     mybir.ActivationFunctionType.Relu,
                                     scale=sb_dinv[:, w:w + 1])
                # transpose x1 -> lhsT for the layer-2 dense matmul
                pt = psum_t.tile([P, P], BF16, tag="tpose")
                nc.tensor.transpose(pt[:], x1[:], sb_ident[:])
                x1T = win_pool.tile([P, P], BF16, tag="x1T")
                nc.scalar.copy(x1T[:], pt[:])
                ph2 = psum_dense.tile([P, F1], F32, tag="dense")
                nc.tensor.matmul(ph2[:, :F2], lhsT=x1T[:], rhs=sb_W2[:],
                                 start=True, stop=True)
                nc.scalar.activation(sb_hs2[:, w, :], ph2[:, :F2],
                                     mybir.ActivationFunctionType.Copy,
                                     scale=sb_dinv[:, w:w + 1])
                k = next(i for i in range(AGC)
                         if AB[i] <= w * P < AB[i + 1])
                a = w * P - AB[k]
                nc.sync.dma_start(out=hs2_locs[k][a:a + rows, :],
                                  in_=sb_hs2[:rows, w, :])

            edge_layer([hs1_fA[:].rearrange("p k f -> (p k) f"),
                        hs1_fB[:].rearrange("p k f -> (p k) f")],
                       F1, BF16, sb_acc1, l1_out)
            if dbg:
                nc.sync.dma_start(
                    out=d_hs2d[:], in_=sb_hs2[:].rearrange("p w f -> p (w f)"))

            # ---- phase 5: chunked all-gather of the layer-2 table ---------
            # AGC window-range collectives: early chunks overlap the tail
            # of layer-1; each is bounced into the Local gather table.
            for k in range(AGC):
                rk = AB[k + 1] - AB[k]
                if cc:
                    nc.gpsimd.collective_compute(
                        "AllGather", mybir.AluOpType.bypass,
                        replica_groups=[list(range(C))],
                        ins=[hs2_locs[k][:].opt()],
                        outs=[hs2_fss[k][:].opt()])
                else:
                    nc.sync.dma_start(out=hs2_fss[k][:rk, :],
                                      in_=hs2_locs[k][:])
                base = C * AB[k]
                nc.sync.dma_start(out=hs2_fl[base:base + C * rk, :],
                                  in_=hs2_fss[k][:])

            # ---- phase 6: layer-2 edges + softmax -------------------------
            # logits are O(1) here (normalized adjacency), so exp is taken
            # without the max-subtraction; sum comes from ACT's accumulator.
            def l2_out(w, rows, pa):
                nc.tensor.matmul(pa[:, :F2], lhsT=sb_ident[:],
                                 rhs=sb_hs2[:, w, :],
                                 start=False, stop=not hasb2)
                if hasb2:
                    nc.tensor.matmul(
                        pa[:, :F2],
                        lhsT=sb_dinvr[:, w * P:(w + 1) * P],
                        rhs=sb_b2[0:1, :], start=False, stop=True)
                ex = win_pool.tile([P, F2], F32, tag="ex")
                ssum = small_pool.tile([P, 1], F32, tag="ssum")
                nc.scalar.activation(ex[:], pa[:, :F2],
                                     mybir.ActivationFunctionType.Exp,
                                     scale=sb_dinv[:, w:w + 1],
                                     accum_out=ssum[:])
                rsum = small_pool.tile([P, 1], F32, tag="rsum")
                nc.vector.reciprocal(rsum[:], ssum[:])
                yw = win_pool.tile([P, F2], F32, tag="yw")
                nc.scalar.activation(yw[:], ex[:],
                                     mybir.ActivationFunctionType.Copy,
                                     scale=rsum[:])
                nc.sync.dma_start(out=d_y[w * P:w * P + rows, :],
                                  in_=yw[:rows, :])

            # single pass over parity-pure tiles; gather rows hold 2 nodes
            tab2 = hs2_fl[:].rearrange("(r two) f -> r (two f)", two=2)
            sts2 = {}
            for t0 in range(0, T2, TB):
                n = min(TB, T2 - t0)
                s = s_pool.tile([P, TB, P], BF16, tag="sbuild")
                nc.vector.tensor_tensor(
                    out=s[:, :n, :],
                    in0=sb_sl2[:, t0:t0 + n].to_broadcast([P, n, P]),
                    in1=sb_iota[:].rearrange(
                        "p (o n) -> p o n", o=1).to_broadcast([P, n, P]),
                    op=mybir.AluOpType.is_equal)
                sts2[t0 // TB] = s
            gts2 = {}
            for w in range(W):
                t0w, t1w = int(offw2[w]), int(offw2[w + 1])
                pa = psum_agg.tile([P, F1], F32, tag="agg")
                for t in range(t0w, t1w):
                    if t % TG == 0:
                        n = min(TG, T2 - t)
                        g = gath_pool.tile([P, TG, 2 * F2], BF16, tag="gath")
                        nc.gpsimd.dma_gather(
                            g[:, :n, :], tab2,
                            sb_si2[:, t * 8:(t + n) * 8],
                            n * P, n * P, 2 * F2,
                            queue_num=(t // TG) % NQ)
                        gts2[t // TG] = g
                    par = int(PAR2[t])
                    nc.tensor.matmul(
                        pa[:, :F2],
                        lhsT=sts2[t // TB][:, t % TB, :],
                        rhs=gts2[t // TG][:, t % TG, par * F2:(par + 1) * F2],
                        start=(t == t0w), stop=False)
                rows = min(P, NS - w * P)
                l2_out(w, rows, pa)

    return {
        "in_names": ["xT", "W1", "W2", "b1r", "b2r", "dinv", "dinvf",
                     "dinvr", "iota", "ident", "srcidx", "dstslot",
                     "srcidx2", "dstslot2"],
        "out_name": "y",
    }


# ---------------------------------------------------------------- frontend

_CACHE = {}


def _build_and_compile(N, H, F1, F2, off2, offw2, PAR2, hasb1, hasb2):
    nc = bacc.Bacc("TRN2", target_bir_lowering=False, debug=False,
                   enable_asserts=False, num_devices=C,
                   num_swdge_queues=NQ)
    meta = build_program(nc, N, H, F1, F2, off2, offw2, PAR2, hasb1, hasb2)
    nc.compile()
    return nc, meta


def prepare_inputs(x, edge_index, W1, b1, W2, b2):
    N, H = x.shape
    F1 = W1.shape[1]
    F2 = W2.shape[1]
    NS = N // C
    W = (NS + P - 1) // P
    NSP = W * P
    NCH = _nch(N)
    NPAD = NCH * P

    src = np.asarray(edge_index[0], dtype=np.int64)
    dst = np.asarray(edge_index[1], dtype=np.int64)
    deg = np.bincount(dst, minlength=N).astype(np.float32) + 1.0
    dinv = (1.0 / np.sqrt(deg)).astype(np.float32)

    off2, T_total, per_core = _schedule(src, dst, N)
    offw2, PAR2, T2, per_core2 = _schedule_l2(src, dst, N)

    iota = np.ascontiguousarray(
        np.tile(np.arange(P, dtype=np.float32), (P, 1)).astype(
            ml_dtypes.bfloat16))
    ident = np.eye(P, dtype=ml_dtypes.bfloat16)
    b1r = np.ascontiguousarray(np.tile(np.asarray(b1, np.float32), (P, 1)))
    b2r = np.ascontiguousarray(np.tile(np.asarray(b2, np.float32), (P, 1)))
    W1f = np.ascontiguousarray(
        np.asarray(W1, np.float32).astype(ml_dtypes.bfloat16))
    W2f = np.ascontiguousarray(
        np.asarray(W2, np.float32).astype(ml_dtypes.bfloat16))
    xbf = np.asarray(x, np.float32).astype(ml_dtypes.bfloat16)

    in_maps = []
    for c in range(C):
        # full node range, rotated so this core's shard comes first
        xfull = np.zeros((NPAD, H), ml_dtypes.bfloat16)
        xfull[:N - c * NS] = xbf[c * NS:]
        xfull[N - c * NS:N] = xbf[:c * NS]
        xT = np.ascontiguousarray(xfull.T)
        dvf = np.ones(NPAD, np.float32)
        dvf[:N - c * NS] = dinv[c * NS:]
        dvf[N - c * NS:N] = dinv[:c * NS]
        dvf = np.ascontiguousarray(dvf.reshape(NCH, P).T)
        dv = np.ones(NSP, np.float32)
        dv[:NS] = dinv[c * NS:(c + 1) * NS]
        dvr = np.ascontiguousarray((1.0 / dv)[None, :])
        dv = np.ascontiguousarray(dv.reshape(W, P).T)
        si, sl = per_core[c]
        si2, sl2 = per_core2[c]
        in_maps.append({
            "xT": xT, "W1": W1f, "W2": W2f, "b1r": b1r, "b2r": b2r,
            "dinv": dv, "dinvf": dvf, "dinvr": dvr,
            "iota": iota, "ident": ident,
            "srcidx": si, "dstslot": sl,
            "srcidx2": si2, "dstslot2": sl2,
        })
    hasb = (bool(np.any(np.asarray(b1))), bool(np.any(np.asarray(b2))))
    return in_maps, (N, H, F1, F2, off2, offw2, PAR2, hasb)


def kernel(x, edge_index, W1, b1, W2, b2, trace=False):
    x = np.asarray(x)
    in_maps, key = prepare_inputs(x, edge_index, W1, b1, W2, b2)
    N, H, F1, F2, off2, offw2, PAR2, hasb = key
    ck = (N, H, F1, F2, off2.tobytes(), offw2.tobytes(), PAR2.tobytes(),
          hasb)
    if ck not in _CACHE:
        _CACHE.clear()
        _CACHE[ck] = _build_and_compile(N, H, F1, F2, off2, offw2, PAR2,
                                        hasb[0], hasb[1])
    nc, meta = _CACHE[ck]
    res = run_bass_kernel_spmd(nc, in_maps, core_ids=list(range(C)),
                               trace=trace)
    y = np.concatenate([res.results[c]["y"] for c in range(C)], axis=0)
    if trace:
        kernel.last_exec_time_ns = res.exec_time_ns
    return y.astype(np.float32)


kernel.last_exec_time_ns = None

